# revision 1
# baseline (speedup 1.0000x reference)
"""Trainium2 Bass kernel for the CSSAM sparse-attention module.

Math (per batch b):
  q_in  = src[b] viewed as [C, L] (L = 64*64 = 4096)               (queries)
  kv[j, l] = featpad[b, j//9, kh + 2*oh - 1, kw + 2*ow - 1]
             where (kh, kw) = divmod(j % 9, 3), l = oh*64 + ow     (keys/vals)
      -> only feat channels 0..28 are ever used (first 256 of C*9 unfold rows)
  Q^T = Wq @ q_in + bq ; K^T = Wk @ kv + bk ; V = (Wk-like conv)   [C, L]
  per head h (8 heads, d = 32): softmax((Qh^T)^T Kh / sqrt(d)) Vh
  out[b] = (Wo @ O^T + (Wo bv + bo)) * src[b]

Sharding: 8 cores = 2 batches x 4 query-chunks of 1024. K/V work is
replicated across the 4 cores of a batch; everything stays on-device.
K^T and V are computed directly from feat as a 9-tap stride-2 convolution
(matmul accumulation over kernel taps with strided SBUF access patterns),
so the im2col "unfold" is never materialized.

Softmax uses no max-subtraction (scores are tiny: |s| < 1 by construction
of the module: w_scale=0.02 projections of unit-normal data).
Denominators come from ones-vector matmuls accumulated alongside PV^T;
the 1/denom row is broadcast back to 32 partitions with a K=1 matmul.
"""

from contextlib import ExitStack

import numpy as np

import concourse.bass as bass
import concourse.mybir as mybir
import concourse.tile as tile

F32 = mybir.dt.float32
F32R = mybir.dt.float32r
BF16 = mybir.dt.bfloat16
AF = mybir.ActivationFunctionType
ALU = mybir.AluOpType

B = 2
C = 256
NH = 8
HD = 32
H = W = 64
L = H * W            # 4096 query / kv positions per batch
HF = WF = 128        # feat spatial
CF = 29              # feat channels actually used by the module
FR = WF + 2          # zero-padded frame width
NCORE = 8
QCHUNK = L // 4      # 1024 queries per core
QN = 256             # attention q sub-chunk (PSUM-bank friendly)
NQC = QCHUNK // QN   # 4
KT = L // 128        # 32 key tiles
SCALE = float(1.0 / np.sqrt(HD))


def build_kernel(nc: bass.Bass):
    featc = nc.declare_dram_parameter("featc", [CF, HF, WF], BF16, isOutput=False)
    srcq = nc.declare_dram_parameter("srcq", [C, QCHUNK], F32, isOutput=False)
    wqt = nc.declare_dram_parameter("wqt", [128, 2, C], F32, isOutput=False)
    wot = nc.declare_dram_parameter("wot", [128, 2, C], F32, isOutput=False)
    wkc = nc.declare_dram_parameter("wkc", [32, 9, C], BF16, isOutput=False)
    wvc = nc.declare_dram_parameter("wvc", [32, 9, C], BF16, isOutput=False)
    bq2 = nc.declare_dram_parameter("bq2", [128, 2], F32, isOutput=False)
    bk2 = nc.declare_dram_parameter("bk2", [128, 2], F32, isOutput=False)
    boe = nc.declare_dram_parameter("boe", [128, 2], F32, isOutput=False)
    onesd = nc.declare_dram_parameter("onesd", [128, 32], BF16, isOutput=False)
    outq = nc.declare_dram_parameter("outq", [C, QCHUNK], F32, isOutput=True)

    with ExitStack() as ctx:
        ctx.enter_context(
            nc.allow_low_precision("float32r tiles carry full fp32 bits")
        )
        tc = ctx.enter_context(tile.TileContext(nc))
        const = ctx.enter_context(tc.tile_pool(name="const", bufs=1))
        convp = ctx.enter_context(tc.tile_pool(name="convp", bufs=1))
        work = ctx.enter_context(tc.tile_pool(name="work", bufs=2))
        pwork = ctx.enter_context(tc.tile_pool(name="pwork", bufs=4))
        psc = ctx.enter_context(tc.tile_pool(name="psc", bufs=2, space="PSUM"))
        pacc = ctx.enter_context(tc.tile_pool(name="pacc", bufs=2, space="PSUM"))

        # ---- constant / input loads ----
        wqt_sb = const.tile([128, 2, C], F32R, tag="wqt")
        nc.sync.dma_start(wqt_sb[:], wqt[:].bitcast(F32R))
        wot_sb = const.tile([128, 2, C], F32R, tag="wot")
        nc.sync.dma_start(wot_sb[:], wot[:].bitcast(F32R))
        wkc_sb = convp.tile([32, 9, C], BF16, tag="wkc")
        nc.sync.dma_start(wkc_sb[:], wkc[:])
        wvc_sb = convp.tile([32, 9, C], BF16, tag="wvc")
        nc.sync.dma_start(wvc_sb[:], wvc[:])
        bq2_sb = const.tile([128, 2], F32, tag="bq2")
        nc.sync.dma_start(bq2_sb[:], bq2[:])
        bk2_sb = const.tile([128, 2], F32, tag="bk2")
        nc.sync.dma_start(bk2_sb[:], bk2[:])
        boe_sb = const.tile([128, 2], F32, tag="boe")
        nc.sync.dma_start(boe_sb[:], boe[:])
        srcq_sb = const.tile([128, 2, QCHUNK], F32R, tag="srcq")
        nc.sync.dma_start(srcq_sb[:], srcq.rearrange("(o p) n -> p o n", p=128).bitcast(F32R))
        srcf_sb = const.tile([128, 2, QCHUNK], F32, tag="srcf")
        nc.sync.dma_start(srcf_sb[:], srcq.rearrange("(o p) n -> p o n", p=128))
        ones_sb = const.tile([128, 32], BF16, tag="ones")
        nc.sync.dma_start(ones_sb[:], onesd[:])

        # feat with a baked zero border (only row 0 / col 0 are ever read
        # out-of-bounds: kh=0,oh=0 and kw=0,ow=0)
        feat_sb = convp.tile([32, FR * FR], BF16, tag="feat")
        feat3 = feat_sb.rearrange("p (r c) -> p r c", c=FR)
        nc.vector.memset(feat3[0:CF, 0:1, :], 0.0)
        nc.vector.memset(feat3[0:CF, :, 0:1], 0.0)
        nc.sync.dma_start(feat3[0:CF, 1 : HF + 1, 1 : WF + 1], featc[:])

        # ---- Q^T = Wq @ src_chunk + bq   -> [C(part, 2 tiles), QCHUNK] ----
        qT_sb = const.tile([128, 2, QCHUNK], BF16, tag="qT")
        for jo in range(2):
            for qn in range(2):
                ps = psc.tile([128, 4 * QN], F32, tag="sc", name=f"q_ps{jo}{qn}")
                ps = ps[:, 0:512]
                for ki in range(2):
                    nc.tensor.matmul(
                        ps[:],
                        (wqt_sb[:, ki, jo * 128 : (jo + 1) * 128]),
                        (srcq_sb[:, ki, qn * 512 : (qn + 1) * 512]),
                        start=(ki == 0),
                        stop=(ki == 1),
                    )
                nc.vector.tensor_scalar_add(
                    qT_sb[:, jo, qn * 512 : (qn + 1) * 512], ps[:], bq2_sb[:, jo : jo + 1]
                )

        # ---- K^T: 9-tap stride-2 conv over feat -> [C(part, 2 tiles), L] ----
        kT_sb = const.tile([128, 2, L], BF16, tag="kT")
        for jo in range(2):
            for ln in range(8):
                ps = psc.tile([128, 4 * QN], F32, tag="sc", name=f"k_ps{jo}{ln}")
                ps = ps[:, 0:512]
                oh0 = ln * 8
                for kk in range(9):
                    kh, kw = divmod(kk, 3)
                    rhs = feat3[
                        0:CF,
                        kh + 2 * oh0 : kh + 2 * oh0 + 16 : 2,
                        kw : kw + 2 * W : 2,
                    ]
                    nc.tensor.matmul(
                        ps[:],
                        (wkc_sb[0:CF, kk, jo * 128 : (jo + 1) * 128]),
                        (rhs),
                        start=(kk == 0),
                        stop=(kk == 8),
                    )
                nc.vector.tensor_scalar_add(
                    kT_sb[:, jo, ln * 512 : (ln + 1) * 512], ps[:], bk2_sb[:, jo : jo + 1]
                )

        # ---- V: same conv, transposed orientation -> [l(part, 32 tiles), C] ----
        v_sb = const.tile([128, KT, C], BF16, tag="v")
        for lt in range(KT):
            ps = psc.tile([128, 4 * QN], F32, tag="sc", name=f"v_ps{lt}")
            for half in range(2):
                oh = 2 * lt + half
                for kk in range(9):
                    kh, kw = divmod(kk, 3)
                    lhsT = feat3[0:CF, kh + 2 * oh, kw : kw + 2 * W : 2]
                    nc.tensor.matmul(
                        ps[64 * half : 64 * half + 64, 0:C],
                        (lhsT),
                        (wvc_sb[0:CF, kk, :]),
                        start=(kk == 0),
                        stop=(kk == 8),
                        tile_position=(0, 64 * half),
                        skip_group_check=True,
                    )
            nc.vector.tensor_copy(v_sb[:, lt, :], ps[:, 0:C])

        # ---- attention over 4 q sub-chunks of 256 ----
        for qc in range(NQC):
            u_ps = [
                pacc.tile([128, 512], F32, tag="uacc", name=f"u{qc}_{i}")[:, 0:QN]
                for i in range(2)
            ]
            d_ps = [
                pacc.tile([128, 512], F32, tag="dacc", name=f"d{qc}_{i}")[:, 0:QN]
                for i in range(2)
            ]
            for kt in range(KT):
                # scores tile t holds row-groups g=2t, 2t+1: bank b <-> one
                # row group (both jo halves share the row slot, so the PE
                # serializes same-bank writes; distinct groups hit distinct
                # banks and run concurrently)
                p_tiles = []
                for t in range(2):
                    sc = psc.tile([128, 4 * QN], F32, tag="sc", name=f"sc{qc}_{kt}_{t}")
                    for g in (2 * t, 2 * t + 1):
                        for jo in range(2):
                            col = (2 * (g % 2) + jo) * QN
                            nc.tensor.matmul(
                                sc[:, col : col + QN],
                                (kT_sb[32 * g : 32 * g + 32, jo, kt * 128 : (kt + 1) * 128]),
                                (qT_sb[32 * g : 32 * g + 32, jo, qc * QN : (qc + 1) * QN]),
                                start=True,
                                stop=True,
                                tile_position=(32 * g, 0),
                                skip_group_check=True,
                            )
                    p_sb = pwork.tile([128, 4 * QN], BF16, tag="p", name=f"p{qc}_{kt}_{t}")
                    nc.scalar.activation(p_sb[:], sc[:], AF.Exp, scale=SCALE)
                    p_tiles.append(p_sb)
                for h in range(NH):
                    g, jo = h % 4, h // 4
                    psl = p_tiles[g // 2][:, (2 * (g % 2) + jo) * QN :][:, 0:QN]
                    nc.tensor.matmul(
                        u_ps[jo][32 * g : 32 * g + 32, :],
                        (v_sb[:, kt, 32 * h : 32 * h + 32]),
                        psl,
                        start=(kt == 0),
                        stop=(kt == KT - 1),
                        tile_position=(0, 32 * g),
                        skip_group_check=True,
                    )
                    nc.tensor.matmul(
                        d_ps[jo][32 * g : 32 * g + 1, :],
                        (ones_sb[:, 0:1]),
                        psl,
                        start=(kt == 0),
                        stop=(kt == KT - 1),
                        tile_position=(0, 32 * g),
                        skip_group_check=True,
                    )

            # normalize: rec = 1/denom rows, broadcast to 32 partitions via
            # K=1 diagonal-packed matmuls, then O^T = U * rec_bcast
            rec_sb = work.tile([128, 2 * QN], F32, tag="rec")
            for jo in range(2):
                for g in range(4):
                    nc.vector.reciprocal(
                        rec_sb[32 * g : 32 * g + 1, jo * QN : (jo + 1) * QN],
                        d_ps[jo][32 * g : 32 * g + 1, :],
                    )
            # split 1/denom into bf16 hi + residual, broadcast to 32
            # partitions with two accumulating diag-packed bf16 matmuls
            rec_hi = work.tile([128, 2 * QN], BF16, tag="rec_hi")
            rec_lo = work.tile([128, 2 * QN], BF16, tag="rec_lo")
            for jo in range(2):
                for g in range(4):
                    r = slice(32 * g, 32 * g + 1)
                    q = slice(jo * QN, (jo + 1) * QN)
                    nc.vector.tensor_copy(rec_hi[r, q], rec_sb[r, q])
                    nc.vector.tensor_sub(rec_lo[r, q], rec_sb[r, q], rec_hi[r, q])
            rb = psc.tile([128, 4 * QN], F32, tag="sc", name=f"rb{qc}")
            for jo in range(2):
                for g in range(4):
                    for part, st in ((rec_hi, True), (rec_lo, False)):
                        nc.tensor.matmul(
                            rb[32 * g : 32 * g + 32, jo * QN : (jo + 1) * QN],
                            ones_sb[32 * g : 32 * g + 1, 0:32],
                            part[32 * g : 32 * g + 1, jo * QN : (jo + 1) * QN],
                            start=st,
                            stop=not st,
                            tile_position=(32 * g, 32 * g),
                            skip_group_check=True,
                        )
            rb_sb = work.tile([128, 2 * QN], F32, tag="rb")
            nc.vector.tensor_copy(rb_sb[:], rb[:, 0 : 2 * QN])
            o_sb = work.tile([128, 2, QN], F32R, tag="o")
            for jo in range(2):
                nc.vector.tensor_tensor(
                    o_sb[:, jo, :],
                    u_ps[jo][:, :],
                    rb_sb[:, jo * QN : (jo + 1) * QN],
                    ALU.mult,
                )

            # out-projection + bias + * src, then store
            for jo in range(2):
                op = pacc.tile([128, 512], F32, tag="dacc", name=f"op{qc}_{jo}")[:, 0:QN]
                for ki in range(2):
                    nc.tensor.matmul(
                        op[:],
                        (wot_sb[:, ki, jo * 128 : (jo + 1) * 128]),
                        (o_sb[:, ki, :]),
                        start=(ki == 0),
                        stop=(ki == 1),
                    )
                ot = work.tile([128, QN], F32, tag="ot")
                nc.vector.tensor_scalar_add(ot[:], op[:], boe_sb[:, jo : jo + 1])
                nc.vector.tensor_tensor(
                    ot[:],
                    ot[:],
                    srcf_sb[:, jo, qc * QN : (qc + 1) * QN],
                    ALU.mult,
                )
                nc.sync.dma_start(
                    outq[jo * 128 : (jo + 1) * 128, qc * QN : (qc + 1) * QN], ot[:]
                )

    return nc


_CACHE: dict = {}


def _split_matmul_waits(nc: bass.Bass):
    """walrus's fp32r self-loading matmul (S3 LW struct) accepts only one
    sync-wait command; peel extra waits onto PE EventSemaphore ops inserted
    immediately before the matmul (same sync point, so no deadlock risk)."""
    import bass_rust

    n_new = 0
    for fn in nc.m.functions:
        for block in fn.blocks:
            insts = list(block.instructions)
            out = []
            changed = False
            skip = (
                mybir.InstEventSemaphore,
                mybir.InstAllEngineBarrier,
                mybir.InstHalt,
            )
            for inst in insts:
                if not isinstance(inst, skip) and inst.sync_info is not None:
                    si = inst.sync_info
                    waits = list(si.on_wait)
                    if len(waits) > 1:
                        for w in waits[:-1]:
                            ev = mybir.InstEventSemaphore(
                                name=f"WSPLIT-{n_new}", ins=[], outs=[]
                            )
                            ev.engine = inst.engine
                            ev.sync_info = bass_rust.SyncInfo(
                                on_wait=[w], on_update=[]
                            )
                            out.append(ev)
                            n_new += 1
                        inst.sync_info = bass_rust.SyncInfo(
                            on_wait=[waits[-1]], on_update=list(si.on_update)
                        )
                        changed = True
                out.append(inst)
            if changed:
                block.instructions = out
    return n_new


def get_nc() -> bass.Bass:
    if "nc" not in _CACHE:
        nc = bass.Bass()
        build_kernel(nc)
        _split_matmul_waits(nc)
        nc.finalize()
        _CACHE["nc"] = nc
    return _CACHE["nc"]


def make_core_inputs(feat, src, Wq, bq, Wk, bk, Wv, bv, Wo, bo):
    """Host-side sharding / layout prep. Returns list of 8 input dicts."""
    f32 = np.float32
    feat = np.asarray(feat, f32)
    src = np.asarray(src, f32)
    Wq, Wk, Wv, Wo = (np.asarray(x, f32) for x in (Wq, Wk, Wv, Wo))
    bq, bk, bv, bo = (np.asarray(x, f32) for x in (bq, bk, bv, bo))

    wqt = np.ascontiguousarray(Wq.T.reshape(2, 128, C).transpose(1, 0, 2))
    wot = np.ascontiguousarray(Wo.T.reshape(2, 128, C).transpose(1, 0, 2))

    # conv-tap layouts: wkc[cp, kk, cout] = Wk[cout, 9*cp + kk] (0 beyond C)
    import ml_dtypes

    bf16 = ml_dtypes.bfloat16
    wkc = np.zeros((32, 9, C), f32)
    wvc = np.zeros((32, 9, C), f32)
    cp_idx, kk_idx = np.meshgrid(np.arange(CF), np.arange(9), indexing="ij")
    j = (9 * cp_idx + kk_idx).ravel()
    valid = j < C
    wkc[cp_idx.ravel()[valid], kk_idx.ravel()[valid], :] = Wk[:, j[valid]].T
    wvc[cp_idx.ravel()[valid], kk_idx.ravel()[valid], :] = Wv[:, j[valid]].T
    wkc = wkc.astype(bf16)
    wvc = wvc.astype(bf16)
    onesd = np.ones((128, 32), bf16)

    bq2 = np.ascontiguousarray(bq.reshape(2, 128).T)
    bk2 = np.ascontiguousarray(bk.reshape(2, 128).T)
    boev = Wo @ bv + bo
    boe = np.ascontiguousarray(boev.reshape(2, 128).T)

    shared = dict(
        wqt=wqt, wot=wot, wkc=wkc, wvc=wvc, bq2=bq2, bk2=bk2, boe=boe, onesd=onesd
    )
    in_maps = []
    for core in range(NCORE):
        b, qi = divmod(core, 4)
        m = dict(shared)
        m["featc"] = np.ascontiguousarray(feat[b, :CF]).astype(bf16)
        m["srcq"] = np.ascontiguousarray(
            src[b].reshape(C, L)[:, qi * QCHUNK : (qi + 1) * QCHUNK]
        )
        in_maps.append(m)
    return in_maps


def _ensure_ntff_hook():
    """Provide antenv.axon_hooks if the image lacks it (needed for trace=True).

    Mirrors trn_agent_boot.trn_boot._ntff_profile_via_ctypes: drives NTFF
    profiling via the axon PJRT .so's C ABI.
    """
    import contextlib
    import ctypes
    import os
    import sys
    import types

    try:
        import antenv.axon_hooks  # noqa: F401

        return
    except ImportError:
        pass

    mod = types.ModuleType("antenv.axon_hooks")
    box = [None]
    mod.set_axon_ntff_profile_hook = lambda h: box.__setitem__(0, h)
    mod.get_axon_ntff_profile_hook = lambda: box[0]
    sys.modules["antenv.axon_hooks"] = mod
    import antenv

    antenv.axon_hooks = mod

    so_path = os.environ.get("PJRT_LIBRARY_PATH", "/opt/axon/libaxon_pjrt.so")
    try:
        lib = ctypes.CDLL(so_path)
    except OSError:
        return
    if not hasattr(lib, "axon_start_nrt_profile"):
        return
    lib.axon_start_nrt_profile.argtypes = [
        ctypes.POINTER(ctypes.c_int64),
        ctypes.c_size_t,
    ]
    lib.axon_start_nrt_profile.restype = ctypes.c_int64
    lib.axon_stop_nrt_profile.argtypes = [ctypes.c_char_p]
    lib.axon_stop_nrt_profile.restype = ctypes.c_int64

    @contextlib.contextmanager
    def _hook(output_dir, device_ids):
        import jax

        jax.devices()
        if device_ids:
            ids = (ctypes.c_int64 * len(device_ids))(*device_ids)
            rc = lib.axon_start_nrt_profile(ids, len(device_ids))
        else:
            rc = lib.axon_start_nrt_profile(None, 0)
        if rc != 0:
            raise RuntimeError(f"axon_start_nrt_profile rc={rc}")
        try:
            yield
        finally:
            n = lib.axon_stop_nrt_profile(str(output_dir).encode())
            print(f"profile: {n} file(s) written to {output_dir}", file=sys.stderr)

    box[0] = _hook


def run(inputs: dict, trace: bool = False, trace_cores=None):
    _ensure_ntff_hook()
    from concourse.bass_utils import run_bass_kernel_spmd

    nc = get_nc()
    in_maps = make_core_inputs(**inputs)
    res = run_bass_kernel_spmd(
        nc,
        in_maps,
        list(range(NCORE)),
        trace=trace,
        trace_cores=trace_cores,
    )
    out = np.empty((B, C, L), np.float32)
    for core in range(NCORE):
        b, qi = divmod(core, 4)
        out[b, :, qi * QCHUNK : (qi + 1) * QCHUNK] = res.results[core]["outq"]
    return out.reshape(B, C, H, W), res


def kernel(feat, src, Wq, bq, Wk, bk, Wv, bv, Wo, bo):
    out, _ = run(
        dict(feat=feat, src=src, Wq=Wq, bq=bq, Wk=Wk, bk=bk, Wv=Wv, bv=bv, Wo=Wo, bo=bo)
    )
    return out



# revision 13
# speedup vs baseline: 1.8249x; 1.8249x over previous
"""Trainium2 Bass kernel for the CSSAM sparse-attention module.

Math (per batch b):
  q_in  = src[b] viewed as [C, L] (L = 64*64 = 4096)               (queries)
  kv[j, l] = featpad[b, j//9, kh + 2*oh - 1, kw + 2*ow - 1]
             where (kh, kw) = divmod(j % 9, 3), l = oh*64 + ow     (keys/vals)
      -> only feat channels 0..28 are ever used (first 256 of C*9 unfold rows)
  Q^T = Wq @ q_in + bq ; K^T = Wk @ kv + bk ; V likewise           [C, L]
  per head h (8 heads, d = 32): softmax((Qh^T)^T Kh / sqrt(d)) Vh
  out[b] = (Wo @ O^T + (Wo bv + bo)) * src[b]

Sharding: 8 cores = 2 batches x 4 query-chunks of 1024. K/V work is
replicated across the 4 cores of a batch; everything stays on-device.

K^T and V come from a 9-tap stride-2 conv over feat. feat is host-prepped
into a phase-split layout featp[32*kw + c, kh%2, r', w'] (stride-2 spatial
phases separated, the 3 kw taps pre-shifted onto partition groups 0/32/64)
so each conv matmul contracts 3 taps at once over contiguous SBUF rows:
3 matmuls per output tile instead of 9, with unit-stride rhs.

Softmax uses no max-subtraction (scores are tiny: |s| < 1 by construction
of the module: w_scale=0.02 projections of unit-normal data).
Denominators ride along as a 33rd all-ones column of V, so P@V and
P@1 come out of one matmul: u-groups are packed 2x(64-aligned) per PSUM
tile (rows 64*(g%2)+0..33, column block g//2). 1/denom rows broadcast to
the 32 dim rows via K=1 f32r matmuls (full fp32 precision, no hi/lo).
"""

from contextlib import ExitStack

import numpy as np

import concourse.bass as bass
import concourse.mybir as mybir
import concourse.tile as tile

F32 = mybir.dt.float32
F32R = mybir.dt.float32r
BF16 = mybir.dt.bfloat16
AF = mybir.ActivationFunctionType
ALU = mybir.AluOpType

B = 2
C = 256
NH = 8
HD = 32
H = W = 64
L = H * W            # 4096 query / kv positions per batch
HF = WF = 128        # feat spatial
CF = 29              # feat channels actually used by the module
NCORE = 8
QCHUNK = L // 4      # 1024 queries per core
QN = 256             # attention q sub-chunk (PSUM-bank friendly)
NQC = QCHUNK // QN   # 4
KT = L // 128        # 32 key tiles
SCALE = float(1.0 / np.sqrt(HD))
FP = 65              # phase-split feat spatial extent


def build_kernel(nc: bass.Bass):
    featp = nc.declare_dram_parameter("featp", [96, 2, FP, FP], BF16, isOutput=False)
    srcq = nc.declare_dram_parameter("srcq", [C, QCHUNK], F32, isOutput=False)
    wqt = nc.declare_dram_parameter("wqt", [128, 2, C], F32, isOutput=False)
    wot = nc.declare_dram_parameter("wot", [128, 2, C], F32, isOutput=False)
    wkp = nc.declare_dram_parameter("wkp", [96, 3, C], BF16, isOutput=False)
    wvp = nc.declare_dram_parameter("wvp", [96, 3, C], BF16, isOutput=False)
    bq2 = nc.declare_dram_parameter("bq2", [128, 2], F32, isOutput=False)
    bk2 = nc.declare_dram_parameter("bk2", [128, 2], F32, isOutput=False)
    boe = nc.declare_dram_parameter("boe", [128, 2], F32, isOutput=False)
    onesd = nc.declare_dram_parameter("onesd", [128, 32], BF16, isOutput=False)
    outq = nc.declare_dram_parameter("outq", [C, QCHUNK], F32, isOutput=True)

    with ExitStack() as ctx:
        ctx.enter_context(
            nc.allow_low_precision("float32r tiles carry full fp32 bits")
        )
        tc = ctx.enter_context(tile.TileContext(nc))
        const = ctx.enter_context(tc.tile_pool(name="const", bufs=1))
        convp = ctx.enter_context(tc.tile_pool(name="convp", bufs=1))
        work = ctx.enter_context(tc.tile_pool(name="work", bufs=2))
        pwork = ctx.enter_context(tc.tile_pool(name="pwork", bufs=4))
        psc = ctx.enter_context(tc.tile_pool(name="psc", bufs=2, space="PSUM"))
        pacc = ctx.enter_context(tc.tile_pool(name="pacc", bufs=2, space="PSUM"))

        # ---- constant / input loads ----
        wqt_sb = const.tile([128, 2, C], F32R, tag="wqt")
        nc.sync.dma_start(wqt_sb[:], wqt[:].bitcast(F32R))
        wot_sb = const.tile([128, 2, C], F32R, tag="wot")
        nc.sync.dma_start(wot_sb[:], wot[:].bitcast(F32R))
        wkp_sb = convp.tile([96, 3, C], BF16, tag="wkp")
        nc.sync.dma_start(wkp_sb[:], wkp[:])
        wvp_sb = convp.tile([96, 3, C], BF16, tag="wvp")
        nc.sync.dma_start(wvp_sb[:], wvp[:])
        bq2_sb = const.tile([128, 2], F32, tag="bq2")
        nc.sync.dma_start(bq2_sb[:], bq2[:])
        bk2_sb = const.tile([128, 2], F32, tag="bk2")
        nc.sync.dma_start(bk2_sb[:], bk2[:])
        boe_sb = const.tile([128, 2], F32, tag="boe")
        nc.sync.dma_start(boe_sb[:], boe[:])
        srcq_sb = const.tile([128, 2, QCHUNK], F32R, tag="srcq")
        nc.sync.dma_start(srcq_sb[:], srcq.rearrange("(o p) n -> p o n", p=128).bitcast(F32R))
        srcf_sb = const.tile([128, 2, QCHUNK], F32, tag="srcf")
        nc.sync.dma_start(srcf_sb[:], srcq.rearrange("(o p) n -> p o n", p=128))
        ones_sb = const.tile([128, 32], BF16, tag="ones")
        nc.sync.dma_start(ones_sb[:], onesd[:])

        # phase-split feat (borders + tap shifts baked on host)
        featp_sb = convp.tile([96, 2, FP, FP], BF16, tag="featp")
        nc.sync.dma_start(featp_sb[:], featp[:])

        # ---- Q^T = Wq @ src_chunk + bq   -> [C(part, 2 tiles), QCHUNK] ----
        qT_sb = const.tile([128, 2, QCHUNK], BF16, tag="qT")
        for jo in range(2):
            for qn in range(2):
                ps = psc.tile([128, 4 * QN], F32, tag="sc", name=f"q_ps{jo}{qn}")
                ps = ps[:, 0:512]
                for ki in range(2):
                    nc.tensor.matmul(
                        ps[:],
                        (wqt_sb[:, ki, jo * 128 : (jo + 1) * 128]),
                        (srcq_sb[:, ki, qn * 512 : (qn + 1) * 512]),
                        start=(ki == 0),
                        stop=(ki == 1),
                    )
                nc.vector.tensor_scalar_add(
                    qT_sb[:, jo, qn * 512 : (qn + 1) * 512], ps[:], bq2_sb[:, jo : jo + 1]
                )

        # ---- K^T: 3-matmul (kh) tap-packed conv -> [C(part, 2 tiles), L] ----
        kT_sb = const.tile([128, 2, L], BF16, tag="kT")
        for jo in range(2):
            for ln in range(8):
                ps = psc.tile([128, 4 * QN], F32, tag="sc", name=f"k_ps{jo}{ln}")
                ps = ps[:, 0:512]
                oh0 = ln * 8
                for kh in range(3):
                    rhs = featp_sb[
                        0:93,
                        kh % 2,
                        kh // 2 + oh0 : kh // 2 + oh0 + 8,
                        0:64,
                    ]
                    nc.tensor.matmul(
                        ps[:],
                        (wkp_sb[0:93, kh, jo * 128 : (jo + 1) * 128]),
                        (rhs),
                        start=(kh == 0),
                        stop=(kh == 2),
                    )
                nc.vector.tensor_scalar_add(
                    kT_sb[:, jo, ln * 512 : (ln + 1) * 512], ps[:], bk2_sb[:, jo : jo + 1]
                )

        # ---- V: same conv, transposed orientation, with a 33rd ones column
        # per head -> v33[l(part, 32 tiles), h, 0:32]=V, [.., 32]=1 ----
        v33_sb = const.tile([128, KT, NH, 33], BF16, tag="v33")
        nc.vector.memset(
            v33_sb.rearrange("p t h d -> p (t h) d")[:, :, 32:33], 1.0
        )
        for lt in range(KT):
            ps = psc.tile([128, 4 * QN], F32, tag="sc", name=f"v_ps{lt}")
            for half in range(2):
                oh = 2 * lt + half
                for kh in range(3):
                    lhsT = featp_sb[0:93, kh % 2, kh // 2 + oh, 0:64]
                    nc.tensor.matmul(
                        ps[64 * half : 64 * half + 64, 0:C],
                        (lhsT),
                        (wvp_sb[0:93, kh, :]),
                        start=(kh == 0),
                        stop=(kh == 2),
                        tile_position=(0, 64 * half),
                        skip_group_check=True,
                    )
            nc.vector.tensor_copy(
                v33_sb[:, lt, :, 0:32],
                ps[:, 0:C].rearrange("p (h d) -> p h d", h=NH),
            )

        # ---- attention over 4 q sub-chunks of 256 ----
        # u tile layout (per jo): rows 64*(g%2)+0..32 = head dims, row
        # 64*(g%2)+32 = denominator; column block (g//2)*QN.
        for qc in range(NQC):
            # column blocks b=0,1 share PSUM banks on the same partitions, so
            # PE start=True zeroing (2KB zero-region granularity) would wipe
            # the sibling block's accumulation: memset + start=False instead
            u_ps = [
                pacc.tile([128, 512], F32, tag="uacc", name=f"u{qc}_{i}")
                for i in range(2)
            ]
            for i in range(2):
                nc.vector.memset(u_ps[i][:], 0.0)
            for kt in range(KT):
                p_tiles = []
                for t in range(2):
                    sc = psc.tile([128, 4 * QN], F32, tag="sc", name=f"sc{qc}_{kt}_{t}")
                    for g in (2 * t, 2 * t + 1):
                        for jo in range(2):
                            col = (2 * (g % 2) + jo) * QN
                            nc.tensor.matmul(
                                sc[:, col : col + QN],
                                (kT_sb[32 * g : 32 * g + 32, jo, kt * 128 : (kt + 1) * 128]),
                                (qT_sb[32 * g : 32 * g + 32, jo, qc * QN : (qc + 1) * QN]),
                                start=True,
                                stop=True,
                                tile_position=(32 * g, 0),
                                skip_group_check=True,
                            )
                    p_sb = pwork.tile([128, 4 * QN], BF16, tag="p", name=f"p{qc}_{kt}_{t}")
                    nc.scalar.activation(p_sb[:], sc[:], AF.Exp, scale=SCALE)
                    p_tiles.append(p_sb)
                for h in range(NH):
                    g, jo = h % 4, h // 4
                    psl = p_tiles[g // 2][:, (2 * (g % 2) + jo) * QN :][:, 0:QN]
                    row = 64 * (g % 2)
                    blk = (g // 2) * QN
                    nc.tensor.matmul(
                        u_ps[jo][row : row + 33, blk : blk + QN],
                        (v33_sb[:, kt, h, :]),
                        psl,
                        start=False,
                        stop=(kt == KT - 1),
                        tile_position=(0, row),
                        skip_group_check=True,
                    )

            # normalize: 1/denom on the two denom rows, split bf16 hi +
            # residual lo, broadcast to the 32 dim rows via two accumulating
            # K=1 matmuls (full fp32 precision reassembled in PSUM)
            rec_sb = work.tile([128, 2, 512], F32, tag="rec")
            for jo in range(2):
                for par in range(2):
                    krow = 64 * par + 32
                    nc.vector.reciprocal(
                        rec_sb[krow : krow + 1, jo, :],
                        u_ps[jo][krow : krow + 1, :],
                    )
            rec_hi = work.tile([128, 2, 512], BF16, tag="rec_hi")
            rec_lo = work.tile([128, 2, 512], BF16, tag="rec_lo")
            for par in range(2):
                krow = 64 * par + 32
                nc.vector.tensor_copy(
                    rec_hi[krow : krow + 1, :, :], rec_sb[krow : krow + 1, :, :]
                )
                nc.vector.tensor_sub(
                    rec_lo[krow : krow + 1, :, :],
                    rec_sb[krow : krow + 1, :, :],
                    rec_hi[krow : krow + 1, :, :],
                )
            rb = psc.tile([128, 4 * QN], F32, tag="sc", name=f"rb{qc}")
            for jo in range(2):
                for par in range(2):
                    krow = 64 * par + 32
                    for part, st in ((rec_hi, True), (rec_lo, False)):
                        nc.tensor.matmul(
                            rb[64 * par : 64 * par + 32, jo * 512 : (jo + 1) * 512],
                            ones_sb[krow : krow + 1, :],
                            part[krow : krow + 1, jo, :],
                            start=st,
                            stop=not st,
                            tile_position=(krow, 64 * par),
                            skip_group_check=True,
                        )
            rb_sb = work.tile([128, 4 * QN], F32, tag="rb")
            nc.vector.tensor_copy(rb_sb[:], rb[:])
            o_sb = work.tile([128, 2, QN], F32R, tag="o")
            for jo in range(2):
                for g in range(4):
                    row = 64 * (g % 2)
                    blk = (g // 2) * QN
                    nc.vector.tensor_tensor(
                        o_sb[32 * g : 32 * g + 32, jo, :],
                        u_ps[jo][row : row + 32, blk : blk + QN],
                        rb_sb[row : row + 32, jo * 512 + blk :][0:32, 0:QN],
                        ALU.mult,
                    )

            # out-projection + bias + * src, then store
            op = pacc.tile([128, 512], F32, tag="op", name=f"op{qc}")
            for jo in range(2):
                opj = op[:, jo * QN : (jo + 1) * QN]
                for ki in range(2):
                    nc.tensor.matmul(
                        opj,
                        (wot_sb[:, ki, jo * 128 : (jo + 1) * 128]),
                        (o_sb[:, ki, :]),
                        start=(ki == 0),
                        stop=(ki == 1),
                    )
                ot = work.tile([128, QN], F32, tag="ot")
                nc.vector.tensor_scalar_add(ot[:], opj, boe_sb[:, jo : jo + 1])
                nc.vector.tensor_tensor(
                    ot[:],
                    ot[:],
                    srcf_sb[:, jo, qc * QN : (qc + 1) * QN],
                    ALU.mult,
                )
                nc.sync.dma_start(
                    outq[jo * 128 : (jo + 1) * 128, qc * QN : (qc + 1) * QN], ot[:]
                )

    return nc


_CACHE: dict = {}


def _split_matmul_waits(nc: bass.Bass):
    """walrus's fp32r self-loading matmul (S3 LW struct) accepts only one
    sync-wait command; peel extra waits onto PE EventSemaphore ops inserted
    immediately before the matmul (same sync point, so no deadlock risk)."""
    import bass_rust

    n_new = 0
    for fn in nc.m.functions:
        for block in fn.blocks:
            insts = list(block.instructions)
            out = []
            changed = False
            skip = (
                mybir.InstEventSemaphore,
                mybir.InstAllEngineBarrier,
                mybir.InstHalt,
            )
            for inst in insts:
                if not isinstance(inst, skip) and inst.sync_info is not None:
                    si = inst.sync_info
                    waits = list(si.on_wait)
                    if len(waits) > 1:
                        for w in waits[:-1]:
                            ev = mybir.InstEventSemaphore(
                                name=f"WSPLIT-{n_new}", ins=[], outs=[]
                            )
                            ev.engine = inst.engine
                            ev.sync_info = bass_rust.SyncInfo(
                                on_wait=[w], on_update=[]
                            )
                            out.append(ev)
                            n_new += 1
                        inst.sync_info = bass_rust.SyncInfo(
                            on_wait=[waits[-1]], on_update=list(si.on_update)
                        )
                        changed = True
                out.append(inst)
            if changed:
                block.instructions = out
    return n_new


def get_nc() -> bass.Bass:
    if "nc" not in _CACHE:
        nc = bass.Bass()
        build_kernel(nc)
        _split_matmul_waits(nc)
        nc.finalize()
        _CACHE["nc"] = nc
    return _CACHE["nc"]


def make_core_inputs(feat, src, Wq, bq, Wk, bk, Wv, bv, Wo, bo):
    """Host-side sharding / layout prep. Returns list of 8 input dicts."""
    f32 = np.float32
    feat = np.asarray(feat, f32)
    src = np.asarray(src, f32)
    Wq, Wk, Wv, Wo = (np.asarray(x, f32) for x in (Wq, Wk, Wv, Wo))
    bq, bk, bv, bo = (np.asarray(x, f32) for x in (bq, bk, bv, bo))

    wqt = np.ascontiguousarray(Wq.T.reshape(2, 128, C).transpose(1, 0, 2))
    wot = np.ascontiguousarray(Wo.T.reshape(2, 128, C).transpose(1, 0, 2))

    import ml_dtypes

    bf16 = ml_dtypes.bfloat16

    # tap-packed conv weights: wkp[32*kw + c, kh, cout] = Wk[cout, 9c+3kh+kw]
    wkp = np.zeros((96, 3, C), f32)
    wvp = np.zeros((96, 3, C), f32)
    for kw in range(3):
        for kh in range(3):
            for c in range(CF):
                j = 9 * c + 3 * kh + kw
                if j < C:
                    wkp[32 * kw + c, kh, :] = Wk[:, j]
                    wvp[32 * kw + c, kh, :] = Wv[:, j]
    wkp = wkp.astype(bf16)
    wvp = wvp.astype(bf16)
    onesd = np.ones((128, 32), bf16)

    bq2 = np.ascontiguousarray(bq.reshape(2, 128).T)
    bk2 = np.ascontiguousarray(bk.reshape(2, 128).T)
    boev = Wo @ bv + bo
    boe = np.ascontiguousarray(boev.reshape(2, 128).T)

    shared = dict(
        wqt=wqt, wot=wot, wkp=wkp, wvp=wvp, bq2=bq2, bk2=bk2, boe=boe, onesd=onesd
    )

    # phase-split feat with the 3 kw taps pre-shifted onto partition groups:
    # featq[c, pr, pc, r', w'] = featpad[c, 2r'+pr, 2w'+pc]
    featp_all = []
    for b in range(B):
        fpad = np.zeros((CF, HF + 2, HF + 2), f32)
        fpad[:, 1 : HF + 1, 1 : HF + 1] = feat[b, :CF]
        featq = (
            fpad[:, : 2 * FP, : 2 * FP]
            .reshape(CF, FP, 2, FP, 2)
            .transpose(0, 2, 4, 1, 3)
        )  # [CF, pr, pc, r', w']
        fp = np.zeros((96, 2, FP, FP), f32)
        fp[0:CF] = featq[:, :, 0]
        fp[32 : 32 + CF] = featq[:, :, 1]
        fp[64 : 64 + CF, :, :, 0 : FP - 1] = featq[:, :, 0, :, 1:FP]
        featp_all.append(fp.astype(bf16))

    in_maps = []
    for core in range(NCORE):
        b, qi = divmod(core, 4)
        m = dict(shared)
        m["featp"] = featp_all[b]
        m["srcq"] = np.ascontiguousarray(
            src[b].reshape(C, L)[:, qi * QCHUNK : (qi + 1) * QCHUNK]
        )
        in_maps.append(m)
    return in_maps


def _ensure_ntff_hook():
    """Provide antenv.axon_hooks if the image lacks it (needed for trace=True).

    Mirrors trn_agent_boot.trn_boot._ntff_profile_via_ctypes: drives NTFF
    profiling via the axon PJRT .so's C ABI.
    """
    import contextlib
    import ctypes
    import os
    import sys
    import types

    try:
        import antenv.axon_hooks  # noqa: F401

        return
    except ImportError:
        pass

    mod = types.ModuleType("antenv.axon_hooks")
    box = [None]
    mod.set_axon_ntff_profile_hook = lambda h: box.__setitem__(0, h)
    mod.get_axon_ntff_profile_hook = lambda: box[0]
    sys.modules["antenv.axon_hooks"] = mod
    import antenv

    antenv.axon_hooks = mod

    so_path = os.environ.get("PJRT_LIBRARY_PATH", "/opt/axon/libaxon_pjrt.so")
    try:
        lib = ctypes.CDLL(so_path)
    except OSError:
        return
    if not hasattr(lib, "axon_start_nrt_profile"):
        return
    lib.axon_start_nrt_profile.argtypes = [
        ctypes.POINTER(ctypes.c_int64),
        ctypes.c_size_t,
    ]
    lib.axon_start_nrt_profile.restype = ctypes.c_int64
    lib.axon_stop_nrt_profile.argtypes = [ctypes.c_char_p]
    lib.axon_stop_nrt_profile.restype = ctypes.c_int64

    @contextlib.contextmanager
    def _hook(output_dir, device_ids):
        import jax

        jax.devices()
        if device_ids:
            ids = (ctypes.c_int64 * len(device_ids))(*device_ids)
            rc = lib.axon_start_nrt_profile(ids, len(device_ids))
        else:
            rc = lib.axon_start_nrt_profile(None, 0)
        if rc != 0:
            raise RuntimeError(f"axon_start_nrt_profile rc={rc}")
        try:
            yield
        finally:
            n = lib.axon_stop_nrt_profile(str(output_dir).encode())
            print(f"profile: {n} file(s) written to {output_dir}", file=sys.stderr)

    box[0] = _hook


def run(inputs: dict, trace: bool = False, trace_cores=None):
    _ensure_ntff_hook()
    from concourse.bass_utils import run_bass_kernel_spmd

    nc = get_nc()
    in_maps = make_core_inputs(**inputs)
    res = run_bass_kernel_spmd(
        nc,
        in_maps,
        list(range(NCORE)),
        trace=trace,
        trace_cores=trace_cores,
    )
    out = np.empty((B, C, L), np.float32)
    for core in range(NCORE):
        b, qi = divmod(core, 4)
        out[b, :, qi * QCHUNK : (qi + 1) * QCHUNK] = res.results[core]["outq"]
    return out.reshape(B, C, H, W), res


def kernel(feat, src, Wq, bq, Wk, bk, Wv, bv, Wo, bo):
    out, _ = run(
        dict(feat=feat, src=src, Wq=Wq, bq=bq, Wk=Wk, bk=bk, Wv=Wv, bv=bv, Wo=Wo, bo=bo)
    )
    return out
